# revision 10
# baseline (speedup 1.0000x reference)
"""Trainium2 Bass kernel for nn_DGLossVer1 (SO(3) gyro loss), bf16 edition.

Math identical to the fp32 baseline: product of 16 (or 32) small-rotation
exponentials via 2nd-order BCH (Z = dt*S + dt^2/2 * C, pairwise tree with
C_AB = C_A + C_B + S_A x S_B), block rotation as unnormalized quaternion
(1, tan(|Z|/2)/|Z| * Z); GT side as true unit quats via rsqrt + Sin table;
rs = log(conj(pred) x gt) with the reference clip semantics.

Perf design:
- Single-region digit-reversed tree, entirely on DVE in bf16 (2x TT mode);
  x,y replica planes built on-chip with TensorCopy. The ~400K-term mean
  absorbs bf16's 0.4% per-value noise (measured end error ~4e-4).
- 3 input DMAs + all output DMAs issued from GpSimd (25ns issue vs 565ns
  on Sync), inputs in 2 chunks so k1 starts during the DMA.
- qmul32 + rel on DVE; GT-exp side on GpSimd+ACT with the
  Abs_reciprocal_sqrt table (4e-5 max rel err; eps guard folded into the
  activation bias); 1/q^2 via reciprocal_approx_fast.
- log phase fp32 from the squares onward (clip + 1-cos^2 need it); the
  0.5*acos(c)/sin coefficient uses a deg-5 Horner fit in |c| (1.7e-5 rel).

Sharding: pure data parallel, 8 sequences per core; each core returns two
partial Huber sums per partition plus the skipped-block rs values; the
host does the tiny weighted reduction (and subtracts the N0 skips).
"""
import numpy as np

P = 128
DT = 0.005
WLOSS = 1.0e6
HUBER = 0.005
N0 = 5
NSEQ, T = 64, 32768
NCORES = 8
SPC = NSEQ // NCORES          # sequences per core
STEPS = SPC * T // P          # 2048 steps per partition
NB16 = STEPS // 16            # 128 16-blocks per partition
NP = STEPS // 2               # 1024 step-pairs per partition
WCOLS = 6 * NP                # host planes: ev x,y,z | od x,y,z (6144)
SKW = 6 * 3 + 5 * 3           # skip outputs per sequence (33)

_CACHE = {}


def _pair_pos(nb):
    """digit-reversed position of logical pair i (n = nb*8)."""
    i = np.arange(nb * 8)
    t = i % 8
    B = i // 8
    t1, t2, t3 = t & 1, (t >> 1) & 1, (t >> 2) & 1
    return (t1 * 4 + t2 * 2 + t3) * nb + (B % 2) * (nb // 2) + B // 2


def _sigma16():
    """logical 16-block index held by dq 16-part column j in [0,128)."""
    pos = _pair_pos(NB16)
    state = np.empty(NP, dtype=np.int64)
    state[pos] = np.arange(NP)
    for _ in range(3):
        state = state[:len(state) // 2] // 2
    return state  # cols of logical {0..4}: 0, 64, 1, 65, 2


def _build(debug=False):
    import concourse.bass as bass
    import concourse.tile as tile
    import concourse.mybir as mybir
    from concourse import bacc

    f32 = mybir.dt.float32
    bf16 = mybir.dt.bfloat16
    AF = mybir.ActivationFunctionType
    OP = mybir.AluOpType
    AX = mybir.AxisListType

    nc = bacc.Bacc(None)
    w_d = nc.declare_dram_parameter("w", [P, WCOLS], bf16, isOutput=False)
    d_d = nc.declare_dram_parameter("d", [P, 3 * NB16], bf16, isOutput=False)
    o_d = nc.declare_dram_parameter("out", [P, 2], f32, isOutput=True)
    skip_d = nc.declare_dram_parameter("skip", [SPC, SKW], f32, isOutput=True)

    CLP = 1.0 - 1e-7

    with tile.TileContext(nc) as tc:
        with tc.tile_pool(name="main", bufs=1) as pool:
            # ---- input DMA: d, then w in 2 half-chunks (GpSimd issue) ----
            wa = pool.tile([P, 10 * NP], bf16)
            d = pool.tile([P, 3 * NB16], bf16)
            wa10 = wa.rearrange("p (k n) -> p k n", k=10)
            wd6 = w_d.rearrange("p (k n) -> p k n", k=6)
            H = NP // 2
            nc.gpsimd.dma_start(wa10[:, 0:3, 0:H], wd6[:, 0:3, 0:H])
            nc.scalar.dma_start(wa10[:, 5:8, 0:H], wd6[:, 3:6, 0:H])
            nc.sync.dma_start(wa10[:, 0:3, H:NP], wd6[:, 0:3, H:NP])
            nc.sync.dma_start(wa10[:, 5:8, H:NP], wd6[:, 3:6, H:NP])
            nc.sync.dma_start(d[:], d_d[:])

            hpi = pool.tile([P, 1], f32)
            nc.gpsimd.memset(hpi[:], float(np.pi / 2))
            epsb = pool.tile([P, 1], f32)
            nc.gpsimd.memset(epsb[:], 1e-30)

            def c3v(t, n, block, off, cnt, nb=3):
                nblocks = t[:].shape[1] // n
                return t.rearrange("p (k n) -> p k n", k=nblocks)[
                    :, block:block + nb, off:off + cnt]

            def repl(wt, lo, hi):
                w10 = wt.rearrange("p (k n) -> p k n", k=10)
                nc.vector.tensor_copy(w10[:, 3:5, lo:hi], w10[:, 0:2, lo:hi])
                nc.vector.tensor_copy(w10[:, 8:10, lo:hi], w10[:, 5:7, lo:hi])

            def k1(wt, npr, S1, C1, CR, lo, hi):
                cnt = hi - lo
                m1 = c3v(C1, npr, 0, lo, cnt)
                nc.vector.tensor_tensor(m1, c3v(wt, npr, 1, lo, cnt),
                                        c3v(wt, npr, 7, lo, cnt), OP.mult)
                m2 = c3v(CR, npr, 0, lo, cnt)
                nc.vector.tensor_tensor(m2, c3v(wt, npr, 2, lo, cnt),
                                        c3v(wt, npr, 6, lo, cnt), OP.mult)
                nc.vector.tensor_tensor(m1, m1, m2, OP.subtract)
                nc.vector.tensor_tensor(c3v(S1, npr, 0, lo, cnt),
                                        c3v(wt, npr, 0, lo, cnt),
                                        c3v(wt, npr, 5, lo, cnt), OP.add)
                nc.vector.tensor_copy(c3v(S1, npr, 3, lo, cnt, 2),
                                      c3v(S1, npr, 0, lo, cnt, 2))

            def level(Sp, Cp, n_in, Sn, Cn, CR, last=False):
                n = n_in // 2
                m1 = c3v(Cn, n, 0, 0, n)
                nc.vector.tensor_tensor(m1, c3v(Sp, n_in, 1, 0, n),
                                        c3v(Sp, n_in, 2, n, n), OP.mult)
                m2 = c3v(CR, n, 0, 0, n)
                nc.vector.tensor_tensor(m2, c3v(Sp, n_in, 2, 0, n),
                                        c3v(Sp, n_in, 1, n, n), OP.mult)
                nc.vector.tensor_tensor(m1, m1, m2, OP.subtract)
                nc.vector.tensor_tensor(m2, c3v(Cp, n_in, 0, 0, n),
                                        c3v(Cp, n_in, 0, n, n), OP.add)
                nc.vector.tensor_tensor(m1, m1, m2, OP.add)
                nc.vector.tensor_tensor(c3v(Sn, n, 0, 0, n),
                                        c3v(Sp, n_in, 0, 0, n),
                                        c3v(Sp, n_in, 0, n, n), OP.add)
                if not last:
                    nc.vector.tensor_copy(c3v(Sn, n, 3, 0, n, 2),
                                          c3v(Sn, n, 0, 0, n, 2))

            S1 = pool.tile([P, 5 * NP], bf16)
            C1 = pool.tile([P, 3 * NP], bf16)
            CR = pool.tile([P, 3 * NP], bf16)
            S2 = pool.tile([P, 5 * 512], bf16)
            C2 = pool.tile([P, 3 * 512], bf16)
            S3 = pool.tile([P, 5 * 256], bf16)
            C3 = pool.tile([P, 3 * 256], bf16)
            S4 = pool.tile([P, 5 * 128], bf16)
            C4 = pool.tile([P, 3 * 128], bf16)
            S5 = pool.tile([P, 3 * 64], bf16)
            C5 = pool.tile([P, 3 * 64], bf16)

            repl(wa, 0, H)
            k1(wa, NP, S1, C1, CR, 0, H)
            repl(wa, H, NP)
            k1(wa, NP, S1, C1, CR, H, NP)
            level(S1, C1, NP, S2, C2, CR)
            level(S2, C2, 512, S3, C3, CR)
            level(S3, C3, 256, S4, C4, CR)
            level(S4, C4, 128, S5, C5, CR, last=True)

            # ---- Z join: 16-part cols [0,128), 32-part cols [128,192) ----
            Z = pool.tile([P, 3 * 192], bf16)
            Z3 = Z.rearrange("p (c n) -> p c n", c=3)
            nc.vector.scalar_tensor_tensor(Z3[:, :, 0:128],
                                           c3v(C4, 128, 0, 0, 128), DT / 2,
                                           c3v(S4, 128, 0, 0, 128),
                                           OP.mult, OP.add)
            nc.vector.scalar_tensor_tensor(Z3[:, :, 128:192], C5[:], DT / 2,
                                           S5[:], OP.mult, OP.add)

            # ---- tan-poly: gh (5-plane) ----
            sqz = pool.tile([P, 3 * 192], bf16)
            nc.vector.tensor_tensor(sqz[:], Z[:], Z[:], OP.mult)
            n2z = pool.tile([P, 192], bf16)
            nc.vector.tensor_tensor(n2z[:], sqz[:, 0:192], sqz[:, 192:384], OP.add)
            nc.vector.tensor_tensor(n2z[:], n2z[:], sqz[:, 384:576], OP.add)
            tp = pool.tile([P, 192], bf16)
            nc.vector.tensor_scalar(tp[:], n2z[:], DT ** 4 / 240, DT ** 2 / 24,
                                    OP.mult, OP.add)
            nc.vector.tensor_tensor(tp[:], tp[:], n2z[:], OP.mult)
            nc.vector.tensor_scalar(tp[:], tp[:], DT, 0.5 * DT, OP.mult, OP.add)
            gh = pool.tile([P, 5 * 192], bf16)
            gh5 = gh.rearrange("p (c n) -> p c n", c=5)
            tpb3 = tp[:].unsqueeze(1).broadcast_to([P, 3, 192])
            tpb2 = tp[:].unsqueeze(1).broadcast_to([P, 2, 192])
            nc.vector.tensor_tensor(gh5[:, 0:3], tpb3, Z3, OP.mult)
            nc.vector.tensor_tensor(gh5[:, 3:5], tpb2, Z3[:, 0:2], OP.mult)

            # ---- d16 exp (rsqrt table + Sin table), GpSimd+ACT ----
            dq = pool.tile([P, 4 * 192], bf16)
            dq4 = dq.rearrange("p (c n) -> p c n", c=4)
            d3 = d.rearrange("p (c j) -> p c j", c=3)
            sqd = pool.tile([P, 3 * NB16], bf16)
            nc.gpsimd.tensor_tensor(sqd[:], d[:], d[:], OP.mult)
            n2d = pool.tile([P, NB16], bf16)
            nc.gpsimd.tensor_tensor(n2d[:], sqd[:, 0:128], sqd[:, 128:256], OP.add)
            nc.gpsimd.tensor_tensor(n2d[:], n2d[:], sqd[:, 256:384], OP.add)
            y1 = pool.tile([P, NB16], bf16)
            nc.scalar.activation(y1[:], n2d[:], AF.Abs_reciprocal_sqrt, bias=epsb[:])
            th = pool.tile([P, NB16], bf16)
            nc.gpsimd.tensor_tensor(th[:], n2d[:], y1[:], OP.mult)
            nc.scalar.activation(dq[:, 0:128], th[:], AF.Sin, bias=hpi[:], scale=-0.5)
            s0 = pool.tile([P, NB16], bf16)
            nc.scalar.activation(s0[:], th[:], AF.Sin, scale=0.5)
            nc.gpsimd.tensor_tensor(s0[:], s0[:], y1[:], OP.mult)
            s0b = s0[:].unsqueeze(1).broadcast_to([P, 3, NB16])
            nc.gpsimd.tensor_tensor(dq4[:, 1:4, 0:128], s0b, d3, OP.mult)

            # ---- d32 = qmul(d16 evens, d16 odds) on DVE ----
            q1r = pool.tile([P, 5 * 64], bf16)
            q2r = pool.tile([P, 5 * 64], bf16)
            q1r5 = q1r.rearrange("p (c n) -> p c n", c=5)
            q2r5 = q2r.rearrange("p (c n) -> p c n", c=5)
            nc.vector.tensor_copy(q1r5[:, 0:3], dq4[:, 1:4, 0:64])
            nc.vector.tensor_copy(q1r5[:, 3:5], dq4[:, 1:3, 0:64])
            nc.vector.tensor_copy(q2r5[:, 0:3], dq4[:, 1:4, 64:128])
            nc.vector.tensor_copy(q2r5[:, 3:5], dq4[:, 1:3, 64:128])
            pp = pool.tile([P, 4 * 64], bf16)
            pp4 = pp.rearrange("p (c n) -> p c n", c=4)
            nc.vector.tensor_tensor(pp4, dq4[:, :, 0:64], dq4[:, :, 64:128],
                                    OP.mult)
            w32 = dq[:, 128:192]
            nc.vector.tensor_tensor(w32, pp[:, 0:64], pp[:, 64:128], OP.subtract)
            nc.vector.tensor_tensor(w32, w32, pp[:, 128:192], OP.subtract)
            nc.vector.tensor_tensor(w32, w32, pp[:, 192:256], OP.subtract)
            w1b = dq[:, 0:64].unsqueeze(1).broadcast_to([P, 3, 64])
            w2b = dq[:, 64:128].unsqueeze(1).broadcast_to([P, 3, 64])
            t1 = pool.tile([P, 3 * 64], bf16)
            t13 = t1.rearrange("p (c n) -> p c n", c=3)
            t2 = pool.tile([P, 3 * 64], bf16)
            t23 = t2.rearrange("p (c n) -> p c n", c=3)
            nc.vector.tensor_tensor(t13, w1b, q2r5[:, 0:3], OP.mult)
            nc.vector.tensor_tensor(t23, w2b, q1r5[:, 0:3], OP.mult)
            nc.vector.tensor_tensor(t1[:], t1[:], t2[:], OP.add)
            cr1 = pool.tile([P, 3 * 64], bf16)
            nc.vector.tensor_tensor(cr1.rearrange("p (c n) -> p c n", c=3),
                                    q1r5[:, 1:4], q2r5[:, 2:5], OP.mult)
            nc.vector.tensor_tensor(t23, q1r5[:, 2:5], q2r5[:, 1:4], OP.mult)
            nc.vector.tensor_tensor(cr1[:], cr1[:], t2[:], OP.subtract)
            nc.vector.tensor_tensor(t1[:], t1[:], cr1[:], OP.add)
            nc.vector.tensor_copy(dq4[:, 1:4, 128:192], t13)

            # ---- rel = conj(1, gh) x dq   (width 192), DVE bf16 ----
            dqv = dq4[:, 1:4, :]
            dqr = pool.tile([P, 5 * 192], bf16)
            dqr5 = dqr.rearrange("p (c n) -> p c n", c=5)
            nc.vector.tensor_copy(dqr[:, 0:576], dq[:, 192:768])
            nc.vector.tensor_copy(dqr[:, 576:960], dq[:, 192:576])
            dm = pool.tile([P, 3 * 192], bf16)
            nc.vector.tensor_tensor(dm.rearrange("p (c n) -> p c n", c=3),
                                    gh5[:, 0:3], dqv, OP.mult)
            rw = pool.tile([P, 192], bf16)
            nc.vector.tensor_tensor(rw[:], dm[:, 0:192], dm[:, 192:384], OP.add)
            nc.vector.tensor_tensor(rw[:], rw[:], dm[:, 384:576], OP.add)
            nc.vector.tensor_tensor(rw[:], rw[:], dq[:, 0:192], OP.add)
            cwb = dq[:, 0:192].unsqueeze(1).broadcast_to([P, 3, 192])
            rv = pool.tile([P, 3 * 192], bf16)
            rv3 = rv.rearrange("p (c n) -> p c n", c=3)
            nc.vector.tensor_tensor(rv3, cwb, gh5[:, 0:3], OP.mult)
            nc.vector.tensor_tensor(rv[:], dq[:, 192:], rv[:], OP.subtract)
            crr = pool.tile([P, 3 * 192], bf16)
            nc.vector.tensor_tensor(crr.rearrange("p (c n) -> p c n", c=3),
                                    gh5[:, 1:4], dqr5[:, 2:5], OP.mult)
            nc.vector.tensor_tensor(dm.rearrange("p (c n) -> p c n", c=3),
                                    gh5[:, 2:5], dqr5[:, 1:4], OP.mult)
            nc.vector.tensor_tensor(crr[:], crr[:], dm[:], OP.subtract)
            nc.vector.tensor_tensor(rv[:], rv[:], crr[:], OP.subtract)

            # ---- log (fp32 from squares onward) ----
            W2 = 192
            sqv = pool.tile([P, 3 * W2], f32)
            nc.scalar.activation(sqv[:], rv[:], AF.Square)
            n2v = pool.tile([P, W2], f32)
            nc.vector.tensor_tensor(n2v[:], sqv[:, 0:192], sqv[:, 192:384], OP.add)
            nc.vector.tensor_tensor(n2v[:], n2v[:], sqv[:, 384:576], OP.add)
            w2t = pool.tile([P, W2], f32)
            nc.scalar.activation(w2t[:], rw[:], AF.Square)
            q2t = pool.tile([P, W2], f32)
            nc.vector.tensor_tensor(q2t[:], w2t[:], n2v[:], OP.add)
            rq = pool.tile([P, W2], f32)
            nc.vector.reciprocal_approx_fast(rq[:], q2t[:])
            cost = pool.tile([P, W2], f32)
            nc.vector.tensor_tensor(cost[:], w2t[:], n2v[:], OP.subtract)
            nc.vector.tensor_tensor(cost[:], cost[:], rq[:], OP.mult)
            nc.vector.tensor_scalar(cost[:], cost[:], CLP, -CLP, OP.min, OP.max)
            c2t = pool.tile([P, W2], f32)
            nc.scalar.activation(c2t[:], cost[:], AF.Square)
            nc.vector.tensor_scalar(c2t[:], c2t[:], -1.0, 1.0, OP.mult, OP.add)
            rs2 = pool.tile([P, W2], f32)
            nc.scalar.activation(rs2[:], c2t[:], AF.Abs_reciprocal_sqrt)
            # F(|c|) = 0.5*acos(|c|)/sin(acos(|c|)): deg-5 Horner in a=|c|
            K5 = [0.7853849420235615, -0.49900465988902176, 0.3800535808218428,
                  -0.2708563016962799, 0.1378553128516594, -0.033441262473293]
            a = pool.tile([P, W2], f32)
            nc.scalar.activation(a[:], cost[:], AF.Abs)
            g2 = pool.tile([P, W2], f32)
            nc.vector.tensor_scalar(g2[:], a[:], K5[5], None, OP.mult)
            for j in (4, 3, 2, 1):
                nc.vector.scalar_tensor_tensor(g2[:], g2[:], K5[j], a[:],
                                               OP.add, OP.mult)
            nc.vector.tensor_scalar(g2[:], g2[:], K5[0], None, OP.add)
            sgn = pool.tile([P, W2], f32)
            nc.scalar.activation(sgn[:], cost[:], AF.Sign)
            u1 = pool.tile([P, W2], f32)
            nc.vector.tensor_scalar(u1[:], sgn[:], float(-np.pi / 4),
                                    float(np.pi / 4), OP.mult, OP.add)
            nc.vector.tensor_tensor(u1[:], u1[:], rs2[:], OP.mult)
            nc.vector.tensor_tensor(g2[:], g2[:], sgn[:], OP.mult)
            cf = pool.tile([P, W2], f32)
            nc.vector.tensor_tensor(cf[:], u1[:], g2[:], OP.add)
            cf2 = pool.tile([P, W2], f32)
            nc.vector.scalar_tensor_tensor(cf2[:], rw[:], 4.0, rq[:],
                                           OP.mult, OP.mult)
            nc.vector.tensor_tensor(cf[:], cf[:], cf2[:], OP.mult)
            rs = pool.tile([P, 3 * W2], f32)
            cfb = cf[:].unsqueeze(1).broadcast_to([P, 3, W2])
            rs3 = rs.rearrange("p (c n) -> p c n", c=3)
            nc.vector.tensor_tensor(rs3, cfb, rv3, OP.mult)

            # ---- skip export + huber + partial sums ----
            rs4 = rs.rearrange("p (c g n) -> p c g n", c=3, g=3)
            nc.scalar.dma_start(skip_d[:, 0:9], rs4[0:P:16, :, 0, 0:3])
            nc.scalar.dma_start(skip_d[:, 9:18], rs4[0:P:16, :, 1, 0:3])
            nc.scalar.dma_start(skip_d[:, 18:33], rs3[0:P:16, :, 128:128 + N0])
            xb = pool.tile([P, 3 * W2], f32)
            nc.scalar.activation(xb[:], rs[:], AF.Abs, scale=1.0 / HUBER)
            mb = pool.tile([P, 3 * W2], f32)
            nc.vector.tensor_scalar(mb[:], xb[:], 1.0, None, OP.min)
            tb = pool.tile([P, 3 * W2], f32)
            nc.vector.scalar_tensor_tensor(tb[:], mb[:], -0.5, xb[:],
                                           OP.mult, OP.add)
            nc.vector.tensor_tensor(tb[:], tb[:], mb[:], OP.mult)
            part = pool.tile([P, 2], f32)
            tb3 = tb.rearrange("p (c n) -> p c n", c=3)
            nc.vector.tensor_reduce(part[:, 0:1], tb3[:, :, 0:128], AX.XY, OP.add)
            nc.vector.tensor_reduce(part[:, 1:2], tb3[:, :, 128:192], AX.XY, OP.add)
            nc.scalar.dma_start(o_d[:], part[:])

            if debug:
                for name, t in [("dbg_Z", Z), ("dbg_gh", gh), ("dbg_dq", dq),
                                ("dbg_rw", rw), ("dbg_rv", rv), ("dbg_rs", rs)]:
                    dd = nc.declare_dram_parameter(name, list(t[:].shape),
                                                   t[:].dtype, isOutput=True)
                    nc.sync.dma_start(dd[:], t[:])

    nc.compile()
    return nc


def _get_nc():
    if "nc" not in _CACHE:
        _CACHE["nc"] = _build()
    return _CACHE["nc"]


def shard_inputs(w_hat, dw_16):
    """full inputs -> list of per-core {'w','d'} maps (permuted bf16 planes)."""
    import ml_dtypes
    bf = ml_dtypes.bfloat16
    pos = _pair_pos(NB16)
    inv = np.empty_like(pos); inv[pos] = np.arange(NP)
    dperm = _sigma16()
    maps = []
    for c in range(NCORES):
        wc = w_hat[c * SPC:(c + 1) * SPC].reshape(P, NP, 2, 3)
        ev, od = wc[:, :, 0][:, inv], wc[:, :, 1][:, inv]   # (P, 1024, 3)
        w = np.concatenate([ev.transpose(0, 2, 1).reshape(P, 3 * NP),
                            od.transpose(0, 2, 1).reshape(P, 3 * NP)], 1)
        dc = dw_16[c * SPC:(c + 1) * SPC, ::16].reshape(P, NB16, 3)
        dc = np.ascontiguousarray(dc[:, dperm]).transpose(0, 2, 1)
        maps.append({"w": np.ascontiguousarray(w).astype(bf),
                     "d": np.ascontiguousarray(dc).reshape(P, 3 * NB16).astype(bf)})
    return maps


def _huber_sum_f32(rs_flat):
    x = (np.abs(rs_flat) * np.float32(1.0 / HUBER)).astype(np.float32)
    m = np.minimum(x, np.float32(1.0))
    t = (m * np.float32(-0.5) + x).astype(np.float32)
    return (m * t).astype(np.float32).sum(dtype=np.float64)


def combine_outputs(outs):
    s16 = 0.0
    s32 = 0.0
    for om in outs:
        o = np.asarray(om["out"], dtype=np.float64)
        s16 += o[:, 0].sum()
        s32 += o[:, 1].sum()
        sk = np.asarray(om["skip"], dtype=np.float32)
        g1 = sk[:, 9:18].reshape(SPC, 3, 3)
        # cols {0,1,2} = logical {0,2,4}; cols {64,65} = logical {1,3}
        sel = np.concatenate([sk[:, 0:9], g1[:, :, 0:2].reshape(SPC, -1)], 1)
        s16 -= _huber_sum_f32(sel)
        s32 -= _huber_sum_f32(sk[:, 18:33])
    c16 = NSEQ * (T // 16 - N0) * 3
    c32 = NSEQ * (T // 32 - N0) * 3
    loss = WLOSS * HUBER ** 2 * (s16 / c16) + WLOSS * HUBER ** 2 * (s32 / c32) / 4.0
    return np.float32(loss)


def kernel(w_hat, dw_16):
    from concourse.bass_utils import run_bass_kernel_spmd

    w_hat = np.asarray(w_hat, dtype=np.float32)
    dw_16 = np.asarray(dw_16, dtype=np.float32)
    nc = _get_nc()
    in_maps = shard_inputs(w_hat, dw_16)
    res = run_bass_kernel_spmd(nc, in_maps, list(range(NCORES)))
    return combine_outputs(res.results)


# revision 11
# speedup vs baseline: 1.0355x; 1.0355x over previous
"""Trainium2 Bass kernel for nn_DGLossVer1 (SO(3) gyro loss), bf16 edition.

Math identical to the fp32 baseline: product of 16 (or 32) small-rotation
exponentials via 2nd-order BCH (Z = dt*S + dt^2/2 * C, pairwise tree with
C_AB = C_A + C_B + S_A x S_B), block rotation as unnormalized quaternion
(1, tan(|Z|/2)/|Z| * Z); GT side as true unit quats via rsqrt + Sin table;
rs = log(conj(pred) x gt) with the reference clip semantics.

Perf design:
- Single-region digit-reversed tree, entirely on DVE in bf16 (2x TT mode);
  x,y replica planes built on-chip with TensorCopy. The ~400K-term mean
  absorbs bf16's 0.4% per-value noise (measured end error ~4e-4).
- 3 input DMAs + all output DMAs issued from GpSimd (25ns issue vs 565ns
  on Sync), inputs in 2 chunks so k1 starts during the DMA.
- qmul32 + rel on DVE; GT-exp side on GpSimd+ACT with the
  Abs_reciprocal_sqrt table (4e-5 max rel err; eps guard folded into the
  activation bias); 1/q^2 via reciprocal_approx_fast.
- log phase fp32 from the squares onward (clip + 1-cos^2 need it); the
  0.5*acos(c)/sin coefficient uses a deg-5 Horner fit in |c| (1.7e-5 rel).

Sharding: pure data parallel, 8 sequences per core; each core returns two
partial Huber sums per partition plus the skipped-block rs values; the
host does the tiny weighted reduction (and subtracts the N0 skips).
"""
import numpy as np

P = 128
DT = 0.005
WLOSS = 1.0e6
HUBER = 0.005
N0 = 5
NSEQ, T = 64, 32768
NCORES = 8
SPC = NSEQ // NCORES          # sequences per core
STEPS = SPC * T // P          # 2048 steps per partition
NB16 = STEPS // 16            # 128 16-blocks per partition
NP = STEPS // 2               # 1024 step-pairs per partition
WCOLS = 6 * NP                # host planes: ev x,y,z | od x,y,z (6144)
SKW = 6 * 3 + 5 * 3           # skip outputs per sequence (33)

_CACHE = {}


def _pair_pos(nb):
    """digit-reversed position of logical pair i (n = nb*8)."""
    i = np.arange(nb * 8)
    t = i % 8
    B = i // 8
    t1, t2, t3 = t & 1, (t >> 1) & 1, (t >> 2) & 1
    return (t1 * 4 + t2 * 2 + t3) * nb + (B % 2) * (nb // 2) + B // 2


def _sigma16():
    """logical 16-block index held by dq 16-part column j in [0,128)."""
    pos = _pair_pos(NB16)
    state = np.empty(NP, dtype=np.int64)
    state[pos] = np.arange(NP)
    for _ in range(3):
        state = state[:len(state) // 2] // 2
    return state  # cols of logical {0..4}: 0, 64, 1, 65, 2


def _build(debug=False):
    import concourse.bass as bass
    import concourse.tile as tile
    import concourse.mybir as mybir
    from concourse import bacc

    f32 = mybir.dt.float32
    bf16 = mybir.dt.bfloat16
    AF = mybir.ActivationFunctionType
    OP = mybir.AluOpType
    AX = mybir.AxisListType

    nc = bacc.Bacc(None)
    w_d = nc.declare_dram_parameter("w", [P, WCOLS], bf16, isOutput=False)
    d_d = nc.declare_dram_parameter("d", [P, 3 * NB16], bf16, isOutput=False)
    o_d = nc.declare_dram_parameter("out", [P, 2], f32, isOutput=True)
    skip_d = nc.declare_dram_parameter("skip", [SPC, SKW], f32, isOutput=True)

    CLP = 1.0 - 1e-7

    with tile.TileContext(nc) as tc:
        with tc.tile_pool(name="main", bufs=1) as pool:
            # ---- input DMA: d, then w in 2 half-chunks (GpSimd issue) ----
            wa = pool.tile([P, 10 * NP], bf16)
            d = pool.tile([P, 3 * NB16], bf16)
            wa10 = wa.rearrange("p (k n) -> p k n", k=10)
            wd6 = w_d.rearrange("p (k n) -> p k n", k=6)
            H = NP // 2
            nc.gpsimd.dma_start(d[:], d_d[:])
            nc.gpsimd.dma_start(wa10[:, 0:3, 0:H], wd6[:, 0:3, 0:H])
            nc.gpsimd.dma_start(wa10[:, 5:8, 0:H], wd6[:, 3:6, 0:H])
            nc.gpsimd.dma_start(wa10[:, 0:3, H:NP], wd6[:, 0:3, H:NP])
            nc.gpsimd.dma_start(wa10[:, 5:8, H:NP], wd6[:, 3:6, H:NP])

            hpi = pool.tile([P, 1], f32)
            nc.gpsimd.memset(hpi[:], float(np.pi / 2))
            epsb = pool.tile([P, 1], f32)
            nc.gpsimd.memset(epsb[:], 1e-30)

            def c3v(t, n, block, off, cnt, nb=3):
                nblocks = t[:].shape[1] // n
                return t.rearrange("p (k n) -> p k n", k=nblocks)[
                    :, block:block + nb, off:off + cnt]

            def repl(wt, lo, hi):
                w10 = wt.rearrange("p (k n) -> p k n", k=10)
                nc.vector.tensor_copy(w10[:, 3:5, lo:hi], w10[:, 0:2, lo:hi])
                nc.vector.tensor_copy(w10[:, 8:10, lo:hi], w10[:, 5:7, lo:hi])

            def k1(wt, npr, S1, C1, CR, lo, hi):
                cnt = hi - lo
                m1 = c3v(C1, npr, 0, lo, cnt)
                nc.vector.tensor_tensor(m1, c3v(wt, npr, 1, lo, cnt),
                                        c3v(wt, npr, 7, lo, cnt), OP.mult)
                m2 = c3v(CR, npr, 0, lo, cnt)
                nc.vector.tensor_tensor(m2, c3v(wt, npr, 2, lo, cnt),
                                        c3v(wt, npr, 6, lo, cnt), OP.mult)
                nc.vector.tensor_tensor(m1, m1, m2, OP.subtract)
                nc.vector.tensor_tensor(c3v(S1, npr, 0, lo, cnt),
                                        c3v(wt, npr, 0, lo, cnt),
                                        c3v(wt, npr, 5, lo, cnt), OP.add)
                nc.vector.tensor_copy(c3v(S1, npr, 3, lo, cnt, 2),
                                      c3v(S1, npr, 0, lo, cnt, 2))

            def level(Sp, Cp, n_in, Sn, Cn, CR, last=False):
                n = n_in // 2
                m1 = c3v(Cn, n, 0, 0, n)
                nc.vector.tensor_tensor(m1, c3v(Sp, n_in, 1, 0, n),
                                        c3v(Sp, n_in, 2, n, n), OP.mult)
                m2 = c3v(CR, n, 0, 0, n)
                nc.vector.tensor_tensor(m2, c3v(Sp, n_in, 2, 0, n),
                                        c3v(Sp, n_in, 1, n, n), OP.mult)
                nc.vector.tensor_tensor(m1, m1, m2, OP.subtract)
                nc.vector.tensor_tensor(m2, c3v(Cp, n_in, 0, 0, n),
                                        c3v(Cp, n_in, 0, n, n), OP.add)
                nc.vector.tensor_tensor(m1, m1, m2, OP.add)
                nc.vector.tensor_tensor(c3v(Sn, n, 0, 0, n),
                                        c3v(Sp, n_in, 0, 0, n),
                                        c3v(Sp, n_in, 0, n, n), OP.add)
                if not last:
                    nc.vector.tensor_copy(c3v(Sn, n, 3, 0, n, 2),
                                          c3v(Sn, n, 0, 0, n, 2))

            S1 = pool.tile([P, 5 * NP], bf16)
            C1 = pool.tile([P, 3 * NP], bf16)
            CR = pool.tile([P, 3 * NP], bf16)
            S2 = pool.tile([P, 5 * 512], bf16)
            C2 = pool.tile([P, 3 * 512], bf16)
            S3 = pool.tile([P, 5 * 256], bf16)
            C3 = pool.tile([P, 3 * 256], bf16)
            S4 = pool.tile([P, 5 * 128], bf16)
            C4 = pool.tile([P, 3 * 128], bf16)
            S5 = pool.tile([P, 3 * 64], bf16)
            C5 = pool.tile([P, 3 * 64], bf16)

            repl(wa, 0, H)
            k1(wa, NP, S1, C1, CR, 0, H)
            repl(wa, H, NP)
            k1(wa, NP, S1, C1, CR, H, NP)
            level(S1, C1, NP, S2, C2, CR)
            level(S2, C2, 512, S3, C3, CR)
            level(S3, C3, 256, S4, C4, CR)
            level(S4, C4, 128, S5, C5, CR, last=True)

            # ---- Z join: 16-part cols [0,128), 32-part cols [128,192) ----
            Z = pool.tile([P, 3 * 192], bf16)
            Z3 = Z.rearrange("p (c n) -> p c n", c=3)
            nc.vector.scalar_tensor_tensor(Z3[:, :, 0:128],
                                           c3v(C4, 128, 0, 0, 128), DT / 2,
                                           c3v(S4, 128, 0, 0, 128),
                                           OP.mult, OP.add)
            nc.vector.scalar_tensor_tensor(Z3[:, :, 128:192], C5[:], DT / 2,
                                           S5[:], OP.mult, OP.add)

            # ---- tan-poly: gh (5-plane) ----
            sqz = pool.tile([P, 3 * 192], bf16)
            nc.vector.tensor_tensor(sqz[:], Z[:], Z[:], OP.mult)
            n2z = pool.tile([P, 192], bf16)
            nc.vector.tensor_tensor(n2z[:], sqz[:, 0:192], sqz[:, 192:384], OP.add)
            nc.vector.tensor_tensor(n2z[:], n2z[:], sqz[:, 384:576], OP.add)
            tp = pool.tile([P, 192], bf16)
            nc.vector.tensor_scalar(tp[:], n2z[:], DT ** 4 / 240, DT ** 2 / 24,
                                    OP.mult, OP.add)
            nc.vector.tensor_tensor(tp[:], tp[:], n2z[:], OP.mult)
            nc.vector.tensor_scalar(tp[:], tp[:], DT, 0.5 * DT, OP.mult, OP.add)
            gh = pool.tile([P, 5 * 192], bf16)
            gh5 = gh.rearrange("p (c n) -> p c n", c=5)
            tpb3 = tp[:].unsqueeze(1).broadcast_to([P, 3, 192])
            tpb2 = tp[:].unsqueeze(1).broadcast_to([P, 2, 192])
            nc.vector.tensor_tensor(gh5[:, 0:3], tpb3, Z3, OP.mult)
            nc.vector.tensor_tensor(gh5[:, 3:5], tpb2, Z3[:, 0:2], OP.mult)

            # ---- d16 exp (rsqrt table + Sin table), GpSimd+ACT ----
            dq = pool.tile([P, 4 * 192], bf16)
            dq4 = dq.rearrange("p (c n) -> p c n", c=4)
            d3 = d.rearrange("p (c j) -> p c j", c=3)
            sqd = pool.tile([P, 3 * NB16], bf16)
            nc.gpsimd.tensor_tensor(sqd[:], d[:], d[:], OP.mult)
            n2d = pool.tile([P, NB16], bf16)
            nc.gpsimd.tensor_tensor(n2d[:], sqd[:, 0:128], sqd[:, 128:256], OP.add)
            nc.gpsimd.tensor_tensor(n2d[:], n2d[:], sqd[:, 256:384], OP.add)
            y1 = pool.tile([P, NB16], bf16)
            nc.scalar.activation(y1[:], n2d[:], AF.Abs_reciprocal_sqrt, bias=epsb[:])
            th = pool.tile([P, NB16], bf16)
            nc.gpsimd.tensor_tensor(th[:], n2d[:], y1[:], OP.mult)
            nc.scalar.activation(dq[:, 0:128], th[:], AF.Sin, bias=hpi[:], scale=-0.5)
            s0 = pool.tile([P, NB16], bf16)
            nc.scalar.activation(s0[:], th[:], AF.Sin, scale=0.5)
            nc.gpsimd.tensor_tensor(s0[:], s0[:], y1[:], OP.mult)
            s0b = s0[:].unsqueeze(1).broadcast_to([P, 3, NB16])
            nc.gpsimd.tensor_tensor(dq4[:, 1:4, 0:128], s0b, d3, OP.mult)

            # ---- d32 = qmul(d16 evens, d16 odds) on DVE ----
            q1r = pool.tile([P, 5 * 64], bf16)
            q2r = pool.tile([P, 5 * 64], bf16)
            q1r5 = q1r.rearrange("p (c n) -> p c n", c=5)
            q2r5 = q2r.rearrange("p (c n) -> p c n", c=5)
            nc.vector.tensor_copy(q1r5[:, 0:3], dq4[:, 1:4, 0:64])
            nc.vector.tensor_copy(q1r5[:, 3:5], dq4[:, 1:3, 0:64])
            nc.vector.tensor_copy(q2r5[:, 0:3], dq4[:, 1:4, 64:128])
            nc.vector.tensor_copy(q2r5[:, 3:5], dq4[:, 1:3, 64:128])
            pp = pool.tile([P, 4 * 64], bf16)
            pp4 = pp.rearrange("p (c n) -> p c n", c=4)
            nc.vector.tensor_tensor(pp4, dq4[:, :, 0:64], dq4[:, :, 64:128],
                                    OP.mult)
            w32 = dq[:, 128:192]
            nc.vector.tensor_tensor(w32, pp[:, 0:64], pp[:, 64:128], OP.subtract)
            nc.vector.tensor_tensor(w32, w32, pp[:, 128:192], OP.subtract)
            nc.vector.tensor_tensor(w32, w32, pp[:, 192:256], OP.subtract)
            w1b = dq[:, 0:64].unsqueeze(1).broadcast_to([P, 3, 64])
            w2b = dq[:, 64:128].unsqueeze(1).broadcast_to([P, 3, 64])
            t1 = pool.tile([P, 3 * 64], bf16)
            t13 = t1.rearrange("p (c n) -> p c n", c=3)
            t2 = pool.tile([P, 3 * 64], bf16)
            t23 = t2.rearrange("p (c n) -> p c n", c=3)
            nc.vector.tensor_tensor(t13, w1b, q2r5[:, 0:3], OP.mult)
            nc.vector.tensor_tensor(t23, w2b, q1r5[:, 0:3], OP.mult)
            nc.vector.tensor_tensor(t1[:], t1[:], t2[:], OP.add)
            cr1 = pool.tile([P, 3 * 64], bf16)
            nc.vector.tensor_tensor(cr1.rearrange("p (c n) -> p c n", c=3),
                                    q1r5[:, 1:4], q2r5[:, 2:5], OP.mult)
            nc.vector.tensor_tensor(t23, q1r5[:, 2:5], q2r5[:, 1:4], OP.mult)
            nc.vector.tensor_tensor(cr1[:], cr1[:], t2[:], OP.subtract)
            nc.vector.tensor_tensor(t1[:], t1[:], cr1[:], OP.add)
            nc.vector.tensor_copy(dq4[:, 1:4, 128:192], t13)

            # ---- rel = conj(1, gh) x dq   (width 192), DVE bf16 ----
            dqv = dq4[:, 1:4, :]
            dqr = pool.tile([P, 5 * 192], bf16)
            dqr5 = dqr.rearrange("p (c n) -> p c n", c=5)
            nc.vector.tensor_copy(dqr[:, 0:576], dq[:, 192:768])
            nc.vector.tensor_copy(dqr[:, 576:960], dq[:, 192:576])
            dm = pool.tile([P, 3 * 192], bf16)
            nc.vector.tensor_tensor(dm.rearrange("p (c n) -> p c n", c=3),
                                    gh5[:, 0:3], dqv, OP.mult)
            rw = pool.tile([P, 192], bf16)
            nc.vector.tensor_tensor(rw[:], dm[:, 0:192], dm[:, 192:384], OP.add)
            nc.vector.tensor_tensor(rw[:], rw[:], dm[:, 384:576], OP.add)
            nc.vector.tensor_tensor(rw[:], rw[:], dq[:, 0:192], OP.add)
            cwb = dq[:, 0:192].unsqueeze(1).broadcast_to([P, 3, 192])
            rv = pool.tile([P, 3 * 192], bf16)
            rv3 = rv.rearrange("p (c n) -> p c n", c=3)
            nc.vector.tensor_tensor(rv3, cwb, gh5[:, 0:3], OP.mult)
            nc.vector.tensor_tensor(rv[:], dq[:, 192:], rv[:], OP.subtract)
            crr = pool.tile([P, 3 * 192], bf16)
            nc.vector.tensor_tensor(crr.rearrange("p (c n) -> p c n", c=3),
                                    gh5[:, 1:4], dqr5[:, 2:5], OP.mult)
            nc.vector.tensor_tensor(dm.rearrange("p (c n) -> p c n", c=3),
                                    gh5[:, 2:5], dqr5[:, 1:4], OP.mult)
            nc.vector.tensor_tensor(crr[:], crr[:], dm[:], OP.subtract)
            nc.vector.tensor_tensor(rv[:], rv[:], crr[:], OP.subtract)

            # ---- log (fp32 from squares onward) ----
            W2 = 192
            sqv = pool.tile([P, 3 * W2], f32)
            nc.scalar.activation(sqv[:], rv[:], AF.Square)
            n2v = pool.tile([P, W2], f32)
            nc.vector.tensor_tensor(n2v[:], sqv[:, 0:192], sqv[:, 192:384], OP.add)
            nc.vector.tensor_tensor(n2v[:], n2v[:], sqv[:, 384:576], OP.add)
            w2t = pool.tile([P, W2], f32)
            nc.scalar.activation(w2t[:], rw[:], AF.Square)
            q2t = pool.tile([P, W2], f32)
            nc.vector.tensor_tensor(q2t[:], w2t[:], n2v[:], OP.add)
            rq = pool.tile([P, W2], f32)
            nc.vector.reciprocal_approx_fast(rq[:], q2t[:])
            cost = pool.tile([P, W2], f32)
            nc.vector.tensor_tensor(cost[:], w2t[:], n2v[:], OP.subtract)
            nc.vector.tensor_tensor(cost[:], cost[:], rq[:], OP.mult)
            nc.vector.tensor_scalar(cost[:], cost[:], CLP, -CLP, OP.min, OP.max)
            c2t = pool.tile([P, W2], f32)
            nc.scalar.activation(c2t[:], cost[:], AF.Square)
            nc.vector.tensor_scalar(c2t[:], c2t[:], -1.0, 1.0, OP.mult, OP.add)
            rs2 = pool.tile([P, W2], f32)
            nc.scalar.activation(rs2[:], c2t[:], AF.Abs_reciprocal_sqrt)
            # F(|c|) = 0.5*acos(|c|)/sin(acos(|c|)): deg-5 Horner in a=|c|
            K5 = [0.7853849420235615, -0.49900465988902176, 0.3800535808218428,
                  -0.2708563016962799, 0.1378553128516594, -0.033441262473293]
            a = pool.tile([P, W2], f32)
            nc.scalar.activation(a[:], cost[:], AF.Abs)
            g2 = pool.tile([P, W2], f32)
            nc.vector.tensor_scalar(g2[:], a[:], K5[5], None, OP.mult)
            for j in (4, 3, 2, 1):
                nc.vector.scalar_tensor_tensor(g2[:], g2[:], K5[j], a[:],
                                               OP.add, OP.mult)
            nc.vector.tensor_scalar(g2[:], g2[:], K5[0], None, OP.add)
            sgn = pool.tile([P, W2], f32)
            nc.scalar.activation(sgn[:], cost[:], AF.Sign)
            u1 = pool.tile([P, W2], f32)
            nc.vector.tensor_scalar(u1[:], sgn[:], float(-np.pi / 4),
                                    float(np.pi / 4), OP.mult, OP.add)
            nc.vector.tensor_tensor(u1[:], u1[:], rs2[:], OP.mult)
            nc.vector.tensor_tensor(g2[:], g2[:], sgn[:], OP.mult)
            cf = pool.tile([P, W2], f32)
            nc.vector.tensor_tensor(cf[:], u1[:], g2[:], OP.add)
            cf2 = pool.tile([P, W2], f32)
            nc.vector.scalar_tensor_tensor(cf2[:], rw[:], 4.0, rq[:],
                                           OP.mult, OP.mult)
            nc.vector.tensor_tensor(cf[:], cf[:], cf2[:], OP.mult)
            rs = pool.tile([P, 3 * W2], f32)
            cfb = cf[:].unsqueeze(1).broadcast_to([P, 3, W2])
            rs3 = rs.rearrange("p (c n) -> p c n", c=3)
            nc.vector.tensor_tensor(rs3, cfb, rv3, OP.mult)

            # ---- skip export + huber + partial sums ----
            rs4 = rs.rearrange("p (c g n) -> p c g n", c=3, g=3)
            nc.gpsimd.dma_start(skip_d[:, 0:9], rs4[0:P:16, :, 0, 0:3])
            nc.gpsimd.dma_start(skip_d[:, 9:18], rs4[0:P:16, :, 1, 0:3])
            nc.gpsimd.dma_start(skip_d[:, 18:33], rs3[0:P:16, :, 128:128 + N0])
            xb = pool.tile([P, 3 * W2], f32)
            nc.scalar.activation(xb[:], rs[:], AF.Abs, scale=1.0 / HUBER)
            mb = pool.tile([P, 3 * W2], f32)
            nc.vector.tensor_scalar(mb[:], xb[:], 1.0, None, OP.min)
            tb = pool.tile([P, 3 * W2], f32)
            nc.vector.scalar_tensor_tensor(tb[:], mb[:], -0.5, xb[:],
                                           OP.mult, OP.add)
            nc.vector.tensor_tensor(tb[:], tb[:], mb[:], OP.mult)
            part = pool.tile([P, 2], f32)
            tb3 = tb.rearrange("p (c n) -> p c n", c=3)
            nc.vector.tensor_reduce(part[:, 0:1], tb3[:, :, 0:128], AX.XY, OP.add)
            nc.vector.tensor_reduce(part[:, 1:2], tb3[:, :, 128:192], AX.XY, OP.add)
            nc.gpsimd.dma_start(o_d[:], part[:])

            if debug:
                for name, t in [("dbg_Z", Z), ("dbg_gh", gh), ("dbg_dq", dq),
                                ("dbg_rw", rw), ("dbg_rv", rv), ("dbg_rs", rs)]:
                    dd = nc.declare_dram_parameter(name, list(t[:].shape),
                                                   t[:].dtype, isOutput=True)
                    nc.sync.dma_start(dd[:], t[:])

    nc.compile()
    return nc


def _get_nc():
    if "nc" not in _CACHE:
        _CACHE["nc"] = _build()
    return _CACHE["nc"]


def shard_inputs(w_hat, dw_16):
    """full inputs -> list of per-core {'w','d'} maps (permuted bf16 planes)."""
    import ml_dtypes
    bf = ml_dtypes.bfloat16
    pos = _pair_pos(NB16)
    inv = np.empty_like(pos); inv[pos] = np.arange(NP)
    dperm = _sigma16()
    maps = []
    for c in range(NCORES):
        wc = w_hat[c * SPC:(c + 1) * SPC].reshape(P, NP, 2, 3)
        ev, od = wc[:, :, 0][:, inv], wc[:, :, 1][:, inv]   # (P, 1024, 3)
        w = np.concatenate([ev.transpose(0, 2, 1).reshape(P, 3 * NP),
                            od.transpose(0, 2, 1).reshape(P, 3 * NP)], 1)
        dc = dw_16[c * SPC:(c + 1) * SPC, ::16].reshape(P, NB16, 3)
        dc = np.ascontiguousarray(dc[:, dperm]).transpose(0, 2, 1)
        maps.append({"w": np.ascontiguousarray(w).astype(bf),
                     "d": np.ascontiguousarray(dc).reshape(P, 3 * NB16).astype(bf)})
    return maps


def _huber_sum_f32(rs_flat):
    x = (np.abs(rs_flat) * np.float32(1.0 / HUBER)).astype(np.float32)
    m = np.minimum(x, np.float32(1.0))
    t = (m * np.float32(-0.5) + x).astype(np.float32)
    return (m * t).astype(np.float32).sum(dtype=np.float64)


def combine_outputs(outs):
    s16 = 0.0
    s32 = 0.0
    for om in outs:
        o = np.asarray(om["out"], dtype=np.float64)
        s16 += o[:, 0].sum()
        s32 += o[:, 1].sum()
        sk = np.asarray(om["skip"], dtype=np.float32)
        g1 = sk[:, 9:18].reshape(SPC, 3, 3)
        # cols {0,1,2} = logical {0,2,4}; cols {64,65} = logical {1,3}
        sel = np.concatenate([sk[:, 0:9], g1[:, :, 0:2].reshape(SPC, -1)], 1)
        s16 -= _huber_sum_f32(sel)
        s32 -= _huber_sum_f32(sk[:, 18:33])
    c16 = NSEQ * (T // 16 - N0) * 3
    c32 = NSEQ * (T // 32 - N0) * 3
    loss = WLOSS * HUBER ** 2 * (s16 / c16) + WLOSS * HUBER ** 2 * (s32 / c32) / 4.0
    return np.float32(loss)


def kernel(w_hat, dw_16):
    from concourse.bass_utils import run_bass_kernel_spmd

    w_hat = np.asarray(w_hat, dtype=np.float32)
    dw_16 = np.asarray(dw_16, dtype=np.float32)
    nc = _get_nc()
    in_maps = shard_inputs(w_hat, dw_16)
    res = run_bass_kernel_spmd(nc, in_maps, list(range(NCORES)))
    return combine_outputs(res.results)
